# revision 3
# baseline (speedup 1.0000x reference)
"""DTranNER CRF loss kernel for Trainium2 (8 NeuronCores, data-parallel over batch).

v3 architecture ("alternating-layout scan"):

Batch (B=256) sharded 8 ways (32 sentences/core).  The pairwise CRF
log-partition runs as two vector chains (fwd 257 mats / bwd 254 mats) in
factored linear space.  Each chain-step alternates between two layouts:

* A-step (n-major): partitions (b, n1), free (n2, p).  DVE tensor_tensor
  (bf16, 2x mode) multiplies the exp'ed stream by the replicated state;
  DVE tensor_reduce folds p (innermost 24) -> frag [(b,n1), n2].
* B-step (p-major): partitions (b, p1), free (n, p2).  The A-step frag is
  consumed IN PLACE (each row (b,p1) already holds its own 6-slice of u);
  after the multiply, SIX accumulating PE matmuls (stationary block-selector
  lhsT, strided rhs column-slices) do BOTH the p2 reduction and the
  4-group partition sum + replication in one PSUM tile.  A scalar-engine
  copy (with folded 1/z renorm scale every RN steps) returns the state to
  SBUF bf16.

A fraction of the multiplies and state copies runs on the otherwise-idle
GpSimd (Pool) engine; the unary CRF chain's elementwise multiply also runs
on Pool, its matvec on the tensor engine.  Gold-path scores are
host-gathered operand values (pure data movement); all arithmetic happens
on device.
"""

import numpy as np
import ml_dtypes
from contextlib import ExitStack

import concourse.bass as bass
import concourse.bacc as bacc
import concourse.tile as tile
from concourse import mybir
from concourse.bass_utils import run_bass_kernel_spmd

FP = mybir.dt.float32
BF = mybir.dt.bfloat16

B, T, K = 256, 512, 24
START, STOP = 22, 23
NCORES = 8
N1, N2 = 4, 6
NF = N2 * K  # 144

AF = mybir.ActivationFunctionType
ALU = mybir.AluOpType
AX = mybir.AxisListType


def build_kernel(BC=32, TT=512, TC=32, RN=16, RU=16, A_POOL=0, B_POOL=0, PACE_MS=0.0, CP_ENG='none', U_ENG='dve', ESUB=8, SBB=3, PSB=2):
    """A_POOL/B_POOL of 16 A-/B-step multiplies run on Pool (rest DVE)."""
    PP = BC * N1           # 128
    NFWD = TT // 2 + 1     # 257 fwd matrices (t = 0..256)
    NBWD = TT - 1 - NFWD   # 254 bwd matrices (t = 510..257)
    SL = TT // 2           # unary slots
    UROW = 64
    CP = 3.8               # pairwise exp pre-scale (exp(x-CP))
    CU = 3.8               # unary exp pre-scale
    NRN = 18               # renorm z slots per pairwise chain

    nc = bacc.Bacc("TRN2", target_bir_lowering=False)
    fwdS = nc.dram_tensor("fwdS", [PP, NFWD * NF], FP, kind="ExternalInput")
    bwdS = nc.dram_tensor("bwdS", [PP, NBWD * NF], FP, kind="ExternalInput")
    winit_rep = nc.dram_tensor("winit_rep", [PP, K], FP, kind="ExternalInput")
    ftp2 = nc.dram_tensor("ftp2", [UROW, SL * BC], FP, kind="ExternalInput")
    eflast = nc.dram_tensor("eflast", [K, BC], FP, kind="ExternalInput")
    transT = nc.dram_tensor("transT", [K, K], FP, kind="ExternalInput")
    transO = nc.dram_tensor("transO", [K, K], FP, kind="ExternalInput")
    gvals = nc.dram_tensor("gvals", [BC, 3 * TT + 4], FP, kind="ExternalInput")
    selw = nc.dram_tensor("selw", [PP, N1 * PP], BF, kind="ExternalInput")
    ssum = nc.dram_tensor("ssum", [PP, PP], BF, kind="ExternalInput")
    nll = nc.dram_tensor("nll", [BC], FP, kind="ExternalOutput")
    scr = nc.dram_tensor("scratch", [4, BC], FP)

    with tile.TileContext(nc) as tc, ExitStack() as ctx:
        sb = ctx.enter_context(tc.tile_pool(name="sb", bufs=SBB))
        big = ctx.enter_context(tc.tile_pool(name="big", bufs=3))
        ebig = ctx.enter_context(tc.tile_pool(name="ebig", bufs=3))
        per = ctx.enter_context(tc.tile_pool(name="per", bufs=1))
        psF = ctx.enter_context(tc.tile_pool(name="psF", bufs=PSB, space="PSUM"))
        psB = ctx.enter_context(tc.tile_pool(name="psB", bufs=PSB, space="PSUM"))
        psU = ctx.enter_context(tc.tile_pool(name="psU", bufs=2, space="PSUM"))
        ps1 = ctx.enter_context(tc.tile_pool(name="ps1", bufs=1, space="PSUM"))

        # ---------------- constants ----------------
        cpb = per.tile([128, 1], FP, tag="cpb")
        nc.vector.memset(cpb[:], -CP)
        cub = per.tile([128, 1], FP, tag="cub")
        nc.vector.memset(cub[:], -CU)
        selw_sb = per.tile([PP, N1 * PP], BF, tag="selw")
        nc.sync.dma_start(out=selw_sb[:], in_=selw[:])
        ssum_sb = per.tile([PP, PP], BF, tag="ssum")
        nc.sync.dma_start(out=ssum_sb[:], in_=ssum[:])

        # Unary stationary weights, block matrix [UROW, UROW]
        uwst1 = per.tile([K, K], FP, tag="uwst1")
        nc.sync.dma_start(out=uwst1[:], in_=transT[:])
        uwst2 = per.tile([UROW, K], FP, tag="uwst2")
        nc.sync.dma_start(out=uwst2[32 : 32 + K, :], in_=transO[:])
        uw = per.tile([UROW, UROW], BF, tag="uw")
        nc.vector.memset(uw[:], 0.0)
        nc.scalar.activation(out=uw[0:K, 0:K], in_=uwst1[:], func=AF.Exp)
        nc.scalar.activation(
            out=uw[32 : 32 + K, 32 : 32 + K], in_=uwst2[32 : 32 + K, :], func=AF.Exp
        )

        uones = per.tile([UROW, 2], BF, tag="uones")
        nc.vector.memset(uones[:], 0.0)
        nc.vector.memset(uones[0:K, 0:1], 1.0)
        nc.vector.memset(uones[32 : 32 + K, 1:2], 1.0)
        usel = per.tile([2, UROW], BF, tag="usel")
        nc.vector.memset(usel[:], 0.0)
        nc.vector.memset(usel[0:1, 0:K], 1.0)
        rowB = sb.tile([1, UROW], BF, tag="rowB")
        nc.vector.memset(rowB[:], 0.0)
        nc.vector.memset(rowB[0:1, 32 : 32 + K], 1.0)
        nc.sync.dma_start(out=usel[1:2, :], in_=rowB[:])
        ones2 = per.tile([2, 1], FP, tag="ones2")
        nc.vector.memset(ones2[:], 1.0)

        # ---------------- unary Ef table (loaded lazily in main loop) ----
        eft = per.tile([UROW, SL * BC], BF, tag="eft")
        EFT_CHUNKS = 4
        cs2 = SL // EFT_CHUNKS
        cstep = cs2 * BC
        def load_eft_chunk(c):
            ftile = big.tile([UROW, cstep], FP, tag="ftp_in")
            nc.sync.dma_start(
                out=ftile[:], in_=ftp2[:, c * cstep : (c + 1) * cstep]
            )
            nc.scalar.activation(
                out=eft[:, c * cstep : (c + 1) * cstep], in_=ftile[:], func=AF.Exp,
                bias=cub[0:UROW, :],
            )

        # ---------------- pairwise state init ----------------
        urepF0 = per.tile([PP, K], BF, tag="urepF0")
        nc.vector.memset(urepF0[:], 0.0)
        nc.vector.memset(urepF0[:, START : START + 1], 1.0)
        wtile = sb.tile([PP, K], FP, tag="wtile")
        nc.sync.dma_start(out=wtile[:], in_=winit_rep[:])
        urepB0 = per.tile([PP, K], BF, tag="urepB0")
        nc.scalar.activation(out=urepB0[:], in_=wtile[:], func=AF.Exp, bias=cpb[0:PP, :])

        zbufF = per.tile([PP, NRN], FP, tag="zbufF")
        nc.vector.memset(zbufF[:], 1.0)
        zbufB = per.tile([PP, NRN], FP, tag="zbufB")
        nc.vector.memset(zbufB[:], 1.0)
        zbufU = per.tile([2, (SL // RU + 2) * BC], FP, tag="zbufU")
        nc.vector.memset(zbufU[:], 1.0)

        # unary state [UROW, BC]
        us0 = per.tile([UROW, BC], BF, tag="us0")
        nc.vector.memset(us0[:], 0.0)
        row1 = sb.tile([1, BC], BF, tag="row1")
        nc.vector.memset(row1[:], 1.0)
        nc.sync.dma_start(out=us0[START : START + 1, :], in_=row1[:])
        tstop = sb.tile([UROW, 1], FP, tag="tstop")
        nc.sync.dma_start(
            out=tstop[32 : 32 + K, :],
            in_=transO[STOP : STOP + 1, :].rearrange("o k -> k o"),
        )
        tstop_e = sb.tile([UROW, 1], BF, tag="tstop_e")
        nc.scalar.activation(out=tstop_e[32 : 32 + K, :], in_=tstop[32 : 32 + K, :], func=AF.Exp)
        nc.vector.tensor_copy(
            out=us0[32 : 32 + K, :], in_=tstop_e[32 : 32 + K, :].broadcast_to([K, BC])
        )
        stU = us0

        # ---------------- chain state ----------------
        stF = {"urep": urepF0, "frag": None, "rz": None}
        stB = {"urep": urepB0, "frag": None, "rz": None}
        ps_pool = {0: psF, 1: psB}
        zb = {0: zbufF, 1: zbufB}
        last_k = {0: NFWD - 1, 1: NBWD - 1}
        ntt = [0, 0]  # per-step-type TT counters

        def pairwise_step(c, st, echunk, m, k):
            e_ap = echunk[:, m * NF : (m + 1) * NF]
            if k % 2 == 0:
                # A-step: free (n2, p); in1 = state (PSUM direct, or SBUF copy)
                pool_tt = (ntt[0] % 16) < A_POOL and CP_ENG != "none"
                ntt[0] += 1
                eng = nc.gpsimd if pool_tt else nc.vector
                prod = sb.tile([PP, N2, K], BF, tag="pA")
                in1 = st["urep"][:, :].unsqueeze(1).broadcast_to([PP, N2, K])
                eng.tensor_tensor(
                    out=prod[:], in0=e_ap.rearrange("q (a b) -> q a b", a=N2),
                    in1=in1, op=ALU.mult,
                )
                frag = sb.tile([PP, N2], BF, tag="fr")
                with nc.allow_low_precision("bf16 CRF inner state"):
                    nc.vector.tensor_reduce(out=frag[:], in_=prod[:], axis=AX.X, op=ALU.add)
                if st.get("rz") is not None:
                    frag2 = sb.tile([PP, N2], BF, tag="fs")
                    nc.vector.tensor_scalar(
                        out=frag2[:], in0=frag[:], scalar1=st["rz"][:], scalar2=None,
                        op0=ALU.mult,
                    )
                    st["rz"] = None
                    frag = frag2
                st["frag"] = frag
            else:
                # B-step: free (n, p2); reduction + group-sum via 6 PE matmuls
                pool_tt = (ntt[1] % 16) < B_POOL
                ntt[1] += 1
                eng = nc.gpsimd if pool_tt else nc.vector
                prod2 = sb.tile([PP, K, N2], BF, tag="pB")
                in1 = st["frag"][:, :].unsqueeze(1).broadcast_to([PP, K, N2])
                eng.tensor_tensor(
                    out=prod2[:], in0=e_ap.rearrange("q (a b) -> q a b", a=K),
                    in1=in1, op=ALU.mult,
                )
                ups = ps_pool[c].tile([PP, K], FP, tag=f"ups{c}")
                for j in range(N2):
                    nc.tensor.matmul(
                        out=ups[:], lhsT=ssum_sb[:], rhs=prod2[:, :, j],
                        start=(j == 0), stop=(j == N2 - 1),
                    )
                renorm = (k + 1) % RN == 0 and k < last_k[c]
                if renorm:
                    slot = (k + 1) // RN
                    nc.vector.tensor_reduce(
                        out=zb[c][:, slot : slot + 1], in_=ups[:], axis=AX.X, op=ALU.add
                    )
                    rz = sb.tile([PP, 1], FP, tag=f"rz{c}")
                    nc.vector.reciprocal(out=rz[:], in_=zb[c][:, slot : slot + 1])
                if CP_ENG == "none":
                    st["urep"] = ups
                    if renorm:
                        st["rz"] = rz
                else:
                    urep = sb.tile([PP, K], BF, tag="urc")
                    if renorm:
                        nc.scalar.activation(out=urep[:], in_=ups[:], func=AF.Copy, scale=rz[:])
                    elif CP_ENG == "act":
                        nc.scalar.activation(out=urep[:], in_=ups[:], func=AF.Copy)
                    else:
                        nc.vector.tensor_copy(out=urep[:], in_=ups[:])
                    st["urep"] = urep

        # ---------------- unary step ----------------
        unary = {"stU": stU, "n": 0}

        u_eng = nc.gpsimd if U_ENG == "pool" else nc.vector

        def unary_step():
            g = unary["n"]
            ef_sl = eft[:, g * BC : (g + 1) * BC]
            us_m = sb.tile([UROW, BC], BF, tag="us_m")
            u_eng.tensor_tensor(out=us_m[:], in0=unary["stU"][:], in1=ef_sl, op=ALU.mult)
            vu_ps = psU.tile([UROW, BC], FP, tag="vu")
            nc.tensor.matmul(out=vu_ps[:], lhsT=uw[:], rhs=us_m[:], start=True, stop=True)
            unary["stU"] = vu_ps
            unary["n"] += 1
            nU = unary["n"]
            if nU % RU == 0 and nU < SL:
                us_c = sb.tile([UROW, BC], BF, tag="us_c")
                nc.vector.tensor_copy(out=us_c[:], in_=vu_ps[:])
                unary["stU"] = us_c
                zu_ps = ps1.tile([2, BC], FP, tag="pmisc")
                nc.tensor.matmul(out=zu_ps[:], lhsT=uones[:], rhs=us_c[:], start=True, stop=True)
                zsl = zbufU[:, (nU // RU) * BC : (nU // RU + 1) * BC]
                nc.vector.tensor_copy(out=zsl, in_=zu_ps[:])
                rzu = sb.tile([2, BC], FP, tag="rzu")
                nc.vector.reciprocal(out=rzu[:], in_=zu_ps[:])
                rzu_b = sb.tile([2, BC], BF, tag="rzu_b")
                nc.vector.tensor_copy(out=rzu_b[:], in_=rzu[:])
                rzu_rep = ps1.tile([UROW, BC], FP, tag="pmisc")
                nc.tensor.matmul(out=rzu_rep[:], lhsT=usel[:], rhs=rzu_b[:], start=True, stop=True)
                rzu_s = sb.tile([UROW, BC], BF, tag="rzu_s")
                nc.vector.tensor_copy(out=rzu_s[:], in_=rzu_rep[:])
                us_sc = sb.tile([UROW, BC], BF, tag="us_s")
                nc.vector.tensor_tensor(out=us_sc[:], in0=us_c[:], in1=rzu_s[:], op=ALU.mult)
                unary["stU"] = us_sc

        # ---------------- main streamed loop ----------------
        load_eft_chunk(0)
        prev_eB = None
        chunk_starts = [0, 4] + list(range(TC, NFWD, TC))
        for it, s0 in enumerate(chunk_starts):
            ntF = min(4 if s0 == 0 else (TC - 4 if s0 == 4 else TC), NFWD - s0)
            ntB = min(ntF, NBWD - s0)
            if 2 <= it <= EFT_CHUNKS:
                load_eft_chunk(it - 1)
            ftileF = big.tile([PP, TC * NF], FP, tag="ftileF")
            nc.sync.dma_start(
                out=ftileF[:, 0 : ntF * NF], in_=fwdS[:, s0 * NF : (s0 + ntF) * NF]
            )
            eF = ebig.tile([PP, TC * NF], BF, tag="eF")
            if ntB > 0:
                ftileB = big.tile([PP, TC * NF], FP, tag="ftileB")
                nc.sync.dma_start(
                    out=ftileB[:, 0 : ntB * NF], in_=bwdS[:, s0 * NF : (s0 + ntB) * NF]
                )
                eB = ebig.tile([PP, TC * NF], BF, tag="eB")
            for e0 in range(0, ntF, ESUB):
                e1 = min(e0 + ESUB, ntF)
                nc.scalar.activation(
                    out=eF[:, e0 * NF : e1 * NF], in_=ftileF[:, e0 * NF : e1 * NF],
                    func=AF.Exp, bias=cpb[0:PP, :],
                )
                if ntB > 0 and e0 < ntB:
                    eb1 = min(e0 + ESUB, ntB)
                    nc.scalar.activation(
                        out=eB[:, e0 * NF : eb1 * NF], in_=ftileB[:, e0 * NF : eb1 * NF],
                        func=AF.Exp, bias=cpb[0:PP, :],
                    )
            for m in range(ntF):
                sF_i = s0 + m
                # bwd chain runs one time-step behind fwd (anti-phase: its
                # A-step overlaps fwd's PE round-trip and vice versa)
                sB_i = sF_i - 1
                with tc.tile_wait_until(sF_i * PACE_MS, enable=PACE_MS > 0):
                    if 0 <= sB_i < NBWD:
                        if m == 0:
                            pairwise_step(1, stB, prev_eB[0], prev_eB[1] - 1, sB_i)
                        else:
                            pairwise_step(1, stB, eB, m - 1, sB_i)
                    pairwise_step(0, stF, eF, m, sF_i)
                    if sF_i < SL:
                        unary_step()
            if ntB > 0:
                prev_eB = (eB, ntB)

        # ---------------- pairwise tails ----------------
        # fwd ended with the extra A-step (k=256): gather frag -> full u (4 MMs)
        upsF = ps1.tile([PP, K], FP, tag="pmisc")
        for g in range(N1):
            nc.tensor.matmul(
                out=upsF[:, g * N2 : (g + 1) * N2],
                lhsT=selw_sb[:, g * PP : (g + 1) * PP],
                rhs=stF["frag"][:], start=True, stop=True,
            )
        urepF_fin = sb.tile([PP, K], BF, tag="urf")
        nc.vector.tensor_copy(out=urepF_fin[:], in_=upsF[:])

        pm = sb.tile([PP, K], FP, tag="pmeet")
        nc.vector.tensor_tensor(out=pm[:], in0=urepF_fin[:], in1=stB["urep"][:], op=ALU.mult)
        qq = sb.tile([PP, 1], FP, tag="qq")
        nc.vector.tensor_reduce(out=qq[:], in_=pm[:], axis=AX.X, op=ALU.add)
        lq = sb.tile([PP, 1], FP, tag="lq")
        nc.scalar.activation(out=lq[:], in_=qq[:], func=AF.Ln)
        lzF = sb.tile([PP, NRN], FP, tag="lzF")
        nc.scalar.activation(out=lzF[:], in_=zbufF[:], func=AF.Ln)
        sF = sb.tile([PP, 1], FP, tag="sF")
        nc.vector.tensor_reduce(out=sF[:], in_=lzF[:], axis=AX.X, op=ALU.add)
        lzB = sb.tile([PP, NRN], FP, tag="lzB")
        nc.scalar.activation(out=lzB[:], in_=zbufB[:], func=AF.Ln)
        sB = sb.tile([PP, 1], FP, tag="sB")
        nc.vector.tensor_reduce(out=sB[:], in_=lzB[:], axis=AX.X, op=ALU.add)
        nc.vector.tensor_tensor(out=lq[:], in0=lq[:], in1=sF[:], op=ALU.add)
        nc.vector.tensor_tensor(out=lq[:], in0=lq[:], in1=sB[:], op=ALU.add)
        nc.vector.tensor_scalar(
            out=lq[:], in0=lq[:], scalar1=CP * (NFWD + NBWD + 1), scalar2=None, op0=ALU.add
        )
        nc.sync.dma_start(
            out=scr[0:1, :],
            in_=lq[:, :].rearrange("(b n) o -> b (n o)", n=N1)[:, 0:1],
        )

        # ---------------- unary tail (as baseline) ----------------
        efl = sb.tile([K, BC], FP, tag="efl")
        nc.sync.dma_start(out=efl[:], in_=eflast[:])
        efl_e = sb.tile([K, BC], BF, tag="efl_e")
        nc.scalar.activation(out=efl_e[:], in_=efl[:], func=AF.Exp)
        ustail = sb.tile([UROW, BC], BF, tag="ustail")
        nc.vector.tensor_copy(out=ustail[:], in_=unary["stU"][:])
        usb_c = sb.tile([K, BC], BF, tag="usb_c")
        nc.sync.dma_start(out=usb_c[:], in_=ustail[32 : 32 + K, :])
        um = sb.tile([K, BC], BF, tag="umeet")
        nc.vector.tensor_tensor(out=um[:], in0=ustail[0:K, :], in1=usb_c[:], op=ALU.mult)
        nc.vector.tensor_tensor(out=um[:], in0=um[:], in1=efl_e[:], op=ALU.mult)
        ones_k = sb.tile([K, 1], BF, tag="ones_k")
        nc.vector.memset(ones_k[:], 1.0)
        au_ps = ps1.tile([1, BC], FP, tag="pmisc")
        nc.tensor.matmul(out=au_ps[:], lhsT=ones_k[:], rhs=um[:], start=True, stop=True)
        lau = sb.tile([1, BC], FP, tag="lau")
        nc.scalar.activation(out=lau[:], in_=au_ps[:], func=AF.Ln)
        NZU = SL // RU + 2
        lzU = sb.tile([2, NZU * BC], FP, tag="lzU")
        nc.scalar.activation(out=lzU[:], in_=zbufU[:, 0 : NZU * BC], func=AF.Ln)
        sU = sb.tile([2, BC], FP, tag="sU")
        nc.vector.tensor_reduce(
            out=sU[:],
            in_=lzU[:].rearrange("a (s b) -> a b s", b=BC),
            axis=AX.X,
            op=ALU.add,
        )
        su_ps = ps1.tile([1, BC], FP, tag="pmisc")
        nc.tensor.matmul(out=su_ps[:], lhsT=ones2[:], rhs=sU[:], start=True, stop=True)
        nc.vector.tensor_tensor(out=lau[:], in0=lau[:], in1=su_ps[:], op=ALU.add)
        nc.vector.tensor_scalar(
            out=lau[:], in0=lau[:], scalar1=CU * (2 * SL), scalar2=None, op0=ALU.add
        )
        nc.sync.dma_start(out=scr[1:2, :], in_=lau[:])

        # ---------------- score + output ----------------
        gv = per.tile([BC, 3 * TT + 4], FP, tag="gv")
        nc.sync.dma_start(out=gv[:], in_=gvals[:])
        sc = sb.tile([BC, 1], FP, tag="sc")
        nc.vector.tensor_reduce(out=sc[:], in_=gv[:], axis=AX.X, op=ALU.add)
        app = sb.tile([BC, 1], FP, tag="app")
        nc.sync.dma_start(out=app[:], in_=scr[0:1, :].rearrange("o b -> b o"))
        alu_ = sb.tile([BC, 1], FP, tag="alu")
        nc.sync.dma_start(out=alu_[:], in_=scr[1:2, :].rearrange("o b -> b o"))
        res = sb.tile([BC, 1], FP, tag="res")
        nc.vector.tensor_tensor(out=res[:], in0=app[:], in1=alu_[:], op=ALU.add)
        nc.vector.tensor_tensor(out=res[:], in0=res[:], in1=sc[:], op=ALU.subtract)
        nc.sync.dma_start(out=nll[:], in_=res[:].rearrange("b o -> (b o)"))

    nc.compile()
    return nc


# ======================= host-side prep =======================

def _slotify(X):
    """X [BC, S, O(out), I(in)] -> per-slot tiles.
    A-slots (even s): rows (b,o1), cols (o2, i).
    B-slots (odd s):  rows (b,i1), cols (o, i2)."""
    BC_, S, _, _ = X.shape
    out = np.empty((BC_ * N1, S, NF), np.float32)
    XA = X[:, 0::2]
    SA = XA.shape[1]
    a = XA.reshape(BC_, SA, N1, N2, K).transpose(0, 2, 1, 3, 4)
    out[:, 0::2] = a.reshape(BC_ * N1, SA, NF)
    XB = X[:, 1::2]
    SB_ = XB.shape[1]
    b = XB.reshape(BC_, SB_, K, N1, N2).transpose(0, 3, 1, 2, 4)
    out[:, 1::2] = b.reshape(BC_ * N1, SB_, NF)
    return out


def prep_core_inputs(feats, fpp, transitions, tags, b0, BC, TT):
    NFWD = TT // 2 + 1
    NBWD = TT - 1 - NFWD
    SL = TT // 2
    fe = feats[b0 : b0 + BC]          # [BC, T, K]
    fp = fpp[b0 : b0 + BC]            # [BC, T, K*K]
    tg = tags[b0 : b0 + BC]           # [BC, T]
    fp4 = fp.reshape(BC, TT, K, K)    # [b, t, n(next), p(prev)]

    # fwd chain: u' = M_t u, M = fp4[:, t], t = 0..NFWD-1 (out=n, in=p)
    fwdS = _slotify(np.ascontiguousarray(fp4[:, 0:NFWD], np.float32))
    fwdS = np.ascontiguousarray(fwdS.reshape(PP8(BC), NFWD * NF))

    # bwd chain: u' = N_s u with N = M_t^T, t = TT-2-s... t = 510..257
    tidx = (TT - 2) - np.arange(NBWD)
    Nmat = fp4[:, tidx].transpose(0, 1, 3, 2)  # [b, s, o=p, i=n]
    bwdS = _slotify(np.ascontiguousarray(Nmat, np.float32))
    bwdS = np.ascontiguousarray(bwdS.reshape(PP8(BC), NBWD * NF))

    w = fp4[:, TT - 1, STOP, :]  # [BC, K]
    winit_rep = np.ascontiguousarray(
        np.repeat(w[:, None, :], N1, axis=1).reshape(BC * N1, K), np.float32
    )

    # unary Ef table (as baseline): fwd rows slot s hold feats[t=s-1] (slot0 zero);
    # bwd rows slot s hold feats[t=TT-1-s]
    H = SL
    ftp2 = np.zeros((SL, 64, BC), np.float32)
    ftp2[1:, 0:K, :] = fe[:, 0 : H - 1].transpose(1, 2, 0)
    ftp2[:, 32 : 32 + K, :] = fe[:, TT - 1 : H - 1 : -1].transpose(1, 2, 0)
    ftp2 = np.ascontiguousarray(ftp2.transpose(1, 0, 2).reshape(64, SL * BC))
    eflast = np.ascontiguousarray(fe[:, H - 1, :].T, np.float32)  # [K, BC]

    # gold-path score operands (gather = data movement; summation on device)
    tgi = np.asarray(tg, np.int64)
    te = np.concatenate([np.full((BC, 1), START, np.int64), tgi,
                         np.full((BC, 1), STOP, np.int64)], axis=1)
    nxt, prv = te[:, 1:], te[:, :-1]
    b_ = np.arange(BC)[:, None]
    t_ = np.arange(TT)[None, :]
    gvals = np.zeros((BC, 3 * TT + 4), np.float32)
    gvals[:, 0 : TT + 1] = transitions[nxt, prv]
    gvals[:, TT + 1 : 2 * TT + 1] = np.take_along_axis(
        fe, tgi[:, :, None], axis=2)[..., 0]
    gvals[:, 2 * TT + 1 : 3 * TT + 1] = fp4[b_, np.minimum(t_, TT - 2),
                                            nxt[:, 0:TT], prv[:, 0:TT]]
    gvals[:, 3 * TT] = fp4[np.arange(BC), TT - 1, STOP, tgi[:, -1]]
    gvals[:, 3 * TT - 1] = fp4[np.arange(BC), TT - 2, nxt[:, TT - 2], prv[:, TT - 2]]

    PP = BC * N1
    selw = np.zeros((PP, N1, PP), np.float32)
    b_idx = np.arange(BC)
    for k in range(N1):
        for n1p in range(N1):
            selw[b_idx * N1 + k, k, b_idx * N1 + n1p] = 1.0
    selw = selw.reshape(PP, N1 * PP).astype(ml_dtypes.bfloat16)

    ssum = np.kron(np.eye(BC, dtype=np.float32), np.ones((N1, N1), np.float32))
    ssum = ssum.astype(ml_dtypes.bfloat16)

    return {
        "fwdS": fwdS,
        "bwdS": bwdS,
        "winit_rep": winit_rep,
        "ftp2": ftp2,
        "eflast": eflast,
        "transT": np.ascontiguousarray(transitions.T, np.float32),
        "transO": np.ascontiguousarray(transitions, np.float32),
        "gvals": gvals,
        "selw": selw,
        "ssum": ssum,
    }


def PP8(BC):
    return BC * N1


_NC_CACHE = {}


def get_nc(BC, TT, TC=12, RN=32, RU=64, A_POOL=0, B_POOL=0, PACE_MS=0.0,
           CP_ENG='none', U_ENG='dve', ESUB=4):
    key = (BC, TT, TC, RN, RU, A_POOL, B_POOL, PACE_MS, CP_ENG, U_ENG, ESUB)
    if key not in _NC_CACHE:
        _NC_CACHE[key] = build_kernel(BC=BC, TT=TT, TC=TC, RN=RN, RU=RU,
                                      A_POOL=A_POOL, B_POOL=B_POOL, PACE_MS=PACE_MS,
                                      CP_ENG=CP_ENG, U_ENG=U_ENG, ESUB=ESUB)
    return _NC_CACHE[key]


def kernel(feats, feats_pp, transitions, tags):
    feats = np.asarray(feats, np.float32)
    feats_pp = np.asarray(feats_pp, np.float32)
    transitions = np.asarray(transitions, np.float32)
    tags_np = np.asarray(tags)

    BC = B // NCORES
    nc = get_nc(BC, T)
    in_maps = [
        prep_core_inputs(feats, feats_pp, transitions, tags_np, c * BC, BC, T)
        for c in range(NCORES)
    ]
    r = run_bass_kernel_spmd(nc, in_maps, list(range(NCORES)))
    out = np.concatenate([r.results[c]["nll"] for c in range(NCORES)])
    return out.astype(np.float32)


if __name__ == "__main__":
    rng = np.random.default_rng(0)
    feats = rng.standard_normal((B, T, K), dtype=np.float32)
    fpp = rng.standard_normal((B, T, K * K), dtype=np.float32)
    tr = rng.standard_normal((K, K), dtype=np.float32)
    tr[START, :] = -100.0
    tr[:, STOP] = -100.0
    tags = rng.integers(0, K - 2, size=(B, T)).astype(np.int32)
    out = kernel(feats, fpp, tr, tags)
    print(out.shape, out[:4])


# revision 9
# speedup vs baseline: 1.0767x; 1.0767x over previous
"""DTranNER CRF loss kernel for Trainium2 (8 NeuronCores, data-parallel over batch).

v3 architecture ("alternating-layout scan"):

Batch (B=256) sharded 8 ways (32 sentences/core).  The pairwise CRF
log-partition runs as two vector chains (fwd 257 mats / bwd 254 mats) in
factored linear space.  Each chain-step alternates between two layouts:

* A-step (n-major): partitions (b, n1), free (n2, p).  DVE tensor_tensor
  (bf16, 2x mode) multiplies the exp'ed stream by the replicated state;
  DVE tensor_reduce folds p (innermost 24) -> frag [(b,n1), n2].
* B-step (p-major): partitions (b, p1), free (n, p2).  The A-step frag is
  consumed IN PLACE (each row (b,p1) already holds its own 6-slice of u);
  after the multiply, SIX accumulating PE matmuls (stationary block-selector
  lhsT, strided rhs column-slices) do BOTH the p2 reduction and the
  4-group partition sum + replication in one PSUM tile.  A scalar-engine
  copy (with folded 1/z renorm scale every RN steps) returns the state to
  SBUF bf16.

A fraction of the multiplies and state copies runs on the otherwise-idle
GpSimd (Pool) engine; the unary CRF chain's elementwise multiply also runs
on Pool, its matvec on the tensor engine.  Gold-path scores are
host-gathered operand values (pure data movement); all arithmetic happens
on device.
"""

import numpy as np
import ml_dtypes
from contextlib import ExitStack

import concourse.bass as bass
import concourse.bacc as bacc
import concourse.tile as tile
from concourse import mybir
from concourse.bass_utils import run_bass_kernel_spmd

FP = mybir.dt.float32
BF = mybir.dt.bfloat16

B, T, K = 256, 512, 24
START, STOP = 22, 23
NCORES = 8
N1, N2 = 4, 6
NF = N2 * K  # 144

AF = mybir.ActivationFunctionType
ALU = mybir.AluOpType
AX = mybir.AxisListType


def build_kernel(BC=32, TT=512, TC=32, RN=16, RU=16, A_POOL=0, B_POOL=0, PACE_MS=0.0, CP_ENG='none', U_ENG='dve', ESUB=8, SBB=3, PSB=2, BGB=3):
    """A_POOL/B_POOL of 16 A-/B-step multiplies run on Pool (rest DVE)."""
    PP = BC * N1           # 128
    NFWD = TT // 2 + 1     # 257 fwd matrices (t = 0..256)
    NBWD = TT - 1 - NFWD   # 254 bwd matrices (t = 510..257)
    SL = TT // 2           # unary slots
    UROW = 64
    CP = 3.8               # pairwise exp pre-scale (exp(x-CP))
    CU = 3.8               # unary exp pre-scale
    NRN = 18               # renorm z slots per pairwise chain

    nc = bacc.Bacc("TRN2", target_bir_lowering=False)
    fwdS = nc.dram_tensor("fwdS", [PP, NFWD * NF], FP, kind="ExternalInput")
    bwdS = nc.dram_tensor("bwdS", [PP, NBWD * NF], FP, kind="ExternalInput")
    winit_rep = nc.dram_tensor("winit_rep", [PP, K], FP, kind="ExternalInput")
    ftp2 = nc.dram_tensor("ftp2", [UROW, SL * BC], FP, kind="ExternalInput")
    eflast = nc.dram_tensor("eflast", [K, BC], FP, kind="ExternalInput")
    transT = nc.dram_tensor("transT", [K, K], FP, kind="ExternalInput")
    transO = nc.dram_tensor("transO", [K, K], FP, kind="ExternalInput")
    gvals = nc.dram_tensor("gvals", [BC, 3 * TT + 4], FP, kind="ExternalInput")
    selw = nc.dram_tensor("selw", [PP, N1 * PP], BF, kind="ExternalInput")
    ssum = nc.dram_tensor("ssum", [PP, PP], BF, kind="ExternalInput")
    ident = nc.dram_tensor("ident", [PP, PP], BF, kind="ExternalInput")
    nll = nc.dram_tensor("nll", [BC], FP, kind="ExternalOutput")
    scr = nc.dram_tensor("scratch", [4, BC], FP)

    with tile.TileContext(nc) as tc, ExitStack() as ctx:
        sb = ctx.enter_context(tc.tile_pool(name="sb", bufs=SBB))
        big = ctx.enter_context(tc.tile_pool(name="big", bufs=BGB))
        ebig = ctx.enter_context(tc.tile_pool(name="ebig", bufs=BGB))
        per = ctx.enter_context(tc.tile_pool(name="per", bufs=1))
        psF = ctx.enter_context(tc.tile_pool(name="psF", bufs=PSB, space="PSUM"))
        psB = ctx.enter_context(tc.tile_pool(name="psB", bufs=PSB, space="PSUM"))
        psU = ctx.enter_context(tc.tile_pool(name="psU", bufs=2, space="PSUM"))
        ps1 = ctx.enter_context(tc.tile_pool(name="ps1", bufs=1, space="PSUM"))

        # ---------------- constants ----------------
        cpb = per.tile([128, 1], FP, tag="cpb")
        nc.vector.memset(cpb[:], -CP)
        cub = per.tile([128, 1], FP, tag="cub")
        nc.vector.memset(cub[:], -CU)
        selw_sb = per.tile([PP, N1 * PP], BF, tag="selw")
        nc.sync.dma_start(out=selw_sb[:], in_=selw[:])
        ssum_sb = per.tile([PP, PP], BF, tag="ssum")
        nc.sync.dma_start(out=ssum_sb[:], in_=ssum[:])
        ident_sb = per.tile([PP, PP], BF, tag="ident")
        nc.sync.dma_start(out=ident_sb[:], in_=ident[:])

        # Unary stationary weights, block matrix [UROW, UROW]
        uwst1 = per.tile([K, K], FP, tag="uwst1")
        nc.sync.dma_start(out=uwst1[:], in_=transT[:])
        uwst2 = per.tile([UROW, K], FP, tag="uwst2")
        nc.sync.dma_start(out=uwst2[32 : 32 + K, :], in_=transO[:])
        uw = per.tile([UROW, UROW], BF, tag="uw")
        nc.vector.memset(uw[:], 0.0)
        nc.scalar.activation(out=uw[0:K, 0:K], in_=uwst1[:], func=AF.Exp)
        nc.scalar.activation(
            out=uw[32 : 32 + K, 32 : 32 + K], in_=uwst2[32 : 32 + K, :], func=AF.Exp
        )

        uones = per.tile([UROW, 2], BF, tag="uones")
        nc.vector.memset(uones[:], 0.0)
        nc.vector.memset(uones[0:K, 0:1], 1.0)
        nc.vector.memset(uones[32 : 32 + K, 1:2], 1.0)
        usel = per.tile([2, UROW], BF, tag="usel")
        nc.vector.memset(usel[:], 0.0)
        nc.vector.memset(usel[0:1, 0:K], 1.0)
        rowB = sb.tile([1, UROW], BF, tag="rowB")
        nc.vector.memset(rowB[:], 0.0)
        nc.vector.memset(rowB[0:1, 32 : 32 + K], 1.0)
        nc.sync.dma_start(out=usel[1:2, :], in_=rowB[:])
        ones2 = per.tile([2, 1], FP, tag="ones2")
        nc.vector.memset(ones2[:], 1.0)

        # ---------------- unary Ef table (loaded lazily in main loop) ----
        eft = per.tile([UROW, SL * BC], BF, tag="eft")
        EFT_CHUNKS = 16
        cs2 = SL // EFT_CHUNKS
        cstep = cs2 * BC
        def load_eft_chunk(c):
            ftile = big.tile([UROW, cstep], FP, tag="ftp_in")
            nc.sync.dma_start(
                out=ftile[:], in_=ftp2[:, c * cstep : (c + 1) * cstep]
            )
            nc.scalar.activation(
                out=eft[:, c * cstep : (c + 1) * cstep], in_=ftile[:], func=AF.Exp,
                bias=cub[0:UROW, :],
            )

        # ---------------- pairwise state init ----------------
        urepF0 = per.tile([PP, K], BF, tag="urepF0")
        nc.vector.memset(urepF0[:], 0.0)
        nc.vector.memset(urepF0[:, START : START + 1], 1.0)
        wtile = sb.tile([PP, K], FP, tag="wtile")
        nc.sync.dma_start(out=wtile[:], in_=winit_rep[:])
        urepB0 = per.tile([PP, K], BF, tag="urepB0")
        nc.scalar.activation(out=urepB0[:], in_=wtile[:], func=AF.Exp, bias=cpb[0:PP, :])

        zbufF = per.tile([PP, NRN], FP, tag="zbufF")
        nc.vector.memset(zbufF[:], 1.0)
        zbufB = per.tile([PP, NRN], FP, tag="zbufB")
        nc.vector.memset(zbufB[:], 1.0)
        zbufU = per.tile([2, (SL // RU + 2) * BC], FP, tag="zbufU")
        nc.vector.memset(zbufU[:], 1.0)

        # unary state [UROW, BC]
        us0 = per.tile([UROW, BC], BF, tag="us0")
        nc.vector.memset(us0[:], 0.0)
        row1 = sb.tile([1, BC], BF, tag="row1")
        nc.vector.memset(row1[:], 1.0)
        nc.sync.dma_start(out=us0[START : START + 1, :], in_=row1[:])
        tstop = sb.tile([UROW, 1], FP, tag="tstop")
        nc.sync.dma_start(
            out=tstop[32 : 32 + K, :],
            in_=transO[STOP : STOP + 1, :].rearrange("o k -> k o"),
        )
        tstop_e = sb.tile([UROW, 1], BF, tag="tstop_e")
        nc.scalar.activation(out=tstop_e[32 : 32 + K, :], in_=tstop[32 : 32 + K, :], func=AF.Exp)
        nc.vector.tensor_copy(
            out=us0[32 : 32 + K, :], in_=tstop_e[32 : 32 + K, :].broadcast_to([K, BC])
        )
        stU = us0

        # ---------------- chain state ----------------
        stF = {"urep": urepF0, "frag": None, "rz": None}
        stB = {"urep": urepB0, "frag": None, "rz": None}
        ps_pool = {0: psF, 1: psB}
        zb = {0: zbufF, 1: zbufB}
        last_k = {0: NFWD - 1, 1: NBWD - 1}
        ntt = [0, 0]  # per-step-type TT counters

        def pairwise_step(c, st, echunk, m, k):
            e_ap = echunk[:, m * NF : (m + 1) * NF]
            if k % 2 == 0:
                # A-step: free (n2, p)
                prod = sb.tile([PP, N2, K], BF, tag="pA")
                in1 = st["urep"][:, :].unsqueeze(1).broadcast_to([PP, N2, K])
                nc.vector.tensor_tensor(
                    out=prod[:], in0=e_ap.rearrange("q (a b) -> q a b", a=N2),
                    in1=in1, op=ALU.mult,
                )
                if k == last_k[c]:
                    # final fwd A-step: reduce on DVE so the gather MMs can
                    # read frag from SBUF
                    frag = sb.tile([PP, N2], BF, tag="fr")
                    with nc.allow_low_precision("bf16 CRF inner state"):
                        nc.vector.tensor_reduce(out=frag[:], in_=prod[:], axis=AX.X, op=ALU.add)
                    if st.get("rz") is not None:
                        frag2 = sb.tile([PP, N2], BF, tag="fs")
                        nc.vector.tensor_scalar(
                            out=frag2[:], in0=frag[:], scalar1=st["rz"][:], scalar2=None,
                            op0=ALU.mult,
                        )
                        st["rz"] = None
                        frag = frag2
                    st["frag"] = frag
                else:
                    # p-reduce as 24 accumulating identity matmuls -> PSUM
                    fps = ps_pool[c].tile([PP, N2], FP, tag=f"fps{c}", bufs=1)
                    for j in range(K):
                        nc.tensor.matmul(
                            out=fps[:], lhsT=ident_sb[:], rhs=prod[:, :, j],
                            start=(j == 0), stop=(j == K - 1),
                        )
                    st["frag"] = fps
            else:
                # B-step: free (n, p2); reduction + group-sum via 6 PE matmuls
                prod2 = sb.tile([PP, K, N2], BF, tag="pB")
                in1 = st["frag"][:, :].unsqueeze(1).broadcast_to([PP, K, N2])
                if st.get("rz") is not None:
                    nc.vector.scalar_tensor_tensor(
                        out=prod2[:], in0=e_ap.rearrange("q (a b) -> q a b", a=K),
                        scalar=st["rz"][:], in1=in1, op0=ALU.mult, op1=ALU.mult,
                    )
                    st["rz"] = None
                else:
                    nc.vector.tensor_tensor(
                        out=prod2[:], in0=e_ap.rearrange("q (a b) -> q a b", a=K),
                        in1=in1, op=ALU.mult,
                    )
                ups = ps_pool[c].tile([PP, K], FP, tag=f"ups{c}", bufs=1)
                for j in range(N2):
                    nc.tensor.matmul(
                        out=ups[:], lhsT=ssum_sb[:], rhs=prod2[:, :, j],
                        start=(j == 0), stop=(j == N2 - 1),
                    )
                renorm = (k + 1) % RN == 0 and k < last_k[c]
                if renorm:
                    slot = (k + 1) // RN
                    nc.vector.tensor_reduce(
                        out=zb[c][:, slot : slot + 1], in_=ups[:], axis=AX.X, op=ALU.add
                    )
                    rz = sb.tile([PP, 1], FP, tag=f"rz{c}")
                    nc.vector.reciprocal(out=rz[:], in_=zb[c][:, slot : slot + 1])
                if CP_ENG == "none":
                    st["urep"] = ups
                    if renorm:
                        st["rz"] = rz
                else:
                    urep = sb.tile([PP, K], BF, tag="urc")
                    if renorm:
                        nc.scalar.activation(out=urep[:], in_=ups[:], func=AF.Copy, scale=rz[:])
                    elif CP_ENG == "act":
                        nc.scalar.activation(out=urep[:], in_=ups[:], func=AF.Copy)
                    else:
                        nc.vector.tensor_copy(out=urep[:], in_=ups[:])
                    st["urep"] = urep

        # ---------------- unary step ----------------
        unary = {"stU": stU, "n": 0}

        u_eng = nc.gpsimd if U_ENG == "pool" else nc.vector

        def unary_step():
            g = unary["n"]
            ef_sl = eft[:, g * BC : (g + 1) * BC]
            us_m = sb.tile([UROW, BC], BF, tag="us_m")
            u_eng.tensor_tensor(out=us_m[:], in0=unary["stU"][:], in1=ef_sl, op=ALU.mult)
            vu_ps = psU.tile([UROW, BC], FP, tag="vu")
            nc.tensor.matmul(out=vu_ps[:], lhsT=uw[:], rhs=us_m[:], start=True, stop=True)
            unary["stU"] = vu_ps
            unary["n"] += 1
            nU = unary["n"]
            if nU % RU == 0 and nU < SL:
                us_c = sb.tile([UROW, BC], BF, tag="us_c")
                nc.vector.tensor_copy(out=us_c[:], in_=vu_ps[:])
                unary["stU"] = us_c
                zu_ps = ps1.tile([2, BC], FP, tag="pmisc")
                nc.tensor.matmul(out=zu_ps[:], lhsT=uones[:], rhs=us_c[:], start=True, stop=True)
                zsl = zbufU[:, (nU // RU) * BC : (nU // RU + 1) * BC]
                nc.vector.tensor_copy(out=zsl, in_=zu_ps[:])
                rzu = sb.tile([2, BC], FP, tag="rzu")
                nc.vector.reciprocal(out=rzu[:], in_=zu_ps[:])
                rzu_b = sb.tile([2, BC], BF, tag="rzu_b")
                nc.vector.tensor_copy(out=rzu_b[:], in_=rzu[:])
                rzu_rep = ps1.tile([UROW, BC], FP, tag="pmisc")
                nc.tensor.matmul(out=rzu_rep[:], lhsT=usel[:], rhs=rzu_b[:], start=True, stop=True)
                rzu_s = sb.tile([UROW, BC], BF, tag="rzu_s")
                nc.vector.tensor_copy(out=rzu_s[:], in_=rzu_rep[:])
                us_sc = sb.tile([UROW, BC], BF, tag="us_s")
                nc.vector.tensor_tensor(out=us_sc[:], in0=us_c[:], in1=rzu_s[:], op=ALU.mult)
                unary["stU"] = us_sc

        # ---------------- main streamed loop ----------------
        load_eft_chunk(0)
        prev_eB = None
        ramp = [4, 8]
        chunk_spans = []
        pos = 0
        for r in ramp:
            chunk_spans.append((pos, r))
            pos += r
        while pos < NFWD:
            chunk_spans.append((pos, min(TC, NFWD - pos)))
            pos += TC
        eft_next = [1]
        for it, (s0, csz) in enumerate(chunk_spans):
            ntF = min(csz, NFWD - s0)
            ntB = min(ntF, NBWD - s0)
            if eft_next[0] < EFT_CHUNKS and s0 >= eft_next[0] * 12:
                load_eft_chunk(eft_next[0])
                eft_next[0] += 1
            ftileF = big.tile([PP, TC * NF], FP, tag="ftileF")
            nc.sync.dma_start(
                out=ftileF[:, 0 : ntF * NF], in_=fwdS[:, s0 * NF : (s0 + ntF) * NF]
            )
            eF = ebig.tile([PP, TC * NF], BF, tag="eF")
            if ntB > 0:
                ftileB = big.tile([PP, TC * NF], FP, tag="ftileB")
                nc.sync.dma_start(
                    out=ftileB[:, 0 : ntB * NF], in_=bwdS[:, s0 * NF : (s0 + ntB) * NF]
                )
                eB = ebig.tile([PP, TC * NF], BF, tag="eB")
            for e0 in range(0, ntF, ESUB):
                e1 = min(e0 + ESUB, ntF)
                nc.scalar.activation(
                    out=eF[:, e0 * NF : e1 * NF], in_=ftileF[:, e0 * NF : e1 * NF],
                    func=AF.Exp, bias=cpb[0:PP, :],
                )
                if ntB > 0 and e0 < ntB:
                    eb1 = min(e0 + ESUB, ntB)
                    nc.scalar.activation(
                        out=eB[:, e0 * NF : eb1 * NF], in_=ftileB[:, e0 * NF : eb1 * NF],
                        func=AF.Exp, bias=cpb[0:PP, :],
                    )
            for m in range(ntF):
                sF_i = s0 + m
                # bwd chain runs one time-step behind fwd (anti-phase: its
                # A-step overlaps fwd's PE round-trip and vice versa)
                sB_i = sF_i - 1
                with tc.tile_wait_until(sF_i * PACE_MS, enable=PACE_MS > 0):
                    if 0 <= sB_i < NBWD:
                        if m == 0:
                            pairwise_step(1, stB, prev_eB[0], prev_eB[1] - 1, sB_i)
                        else:
                            pairwise_step(1, stB, eB, m - 1, sB_i)
                    pairwise_step(0, stF, eF, m, sF_i)
                    if sF_i < SL:
                        unary_step()
            if ntB > 0:
                prev_eB = (eB, ntB)

        # ---------------- pairwise tails ----------------
        # fwd ended with the extra A-step (k=256): gather frag -> full u (4 MMs)
        upsF = ps1.tile([PP, K], FP, tag="pmisc")
        for g in range(N1):
            nc.tensor.matmul(
                out=upsF[:, g * N2 : (g + 1) * N2],
                lhsT=selw_sb[:, g * PP : (g + 1) * PP],
                rhs=stF["frag"][:], start=True, stop=True,
            )
        urepF_fin = sb.tile([PP, K], BF, tag="urf")
        nc.vector.tensor_copy(out=urepF_fin[:], in_=upsF[:])

        pm = sb.tile([PP, K], FP, tag="pmeet")
        nc.vector.tensor_tensor(out=pm[:], in0=urepF_fin[:], in1=stB["urep"][:], op=ALU.mult)
        qq = sb.tile([PP, 1], FP, tag="qq")
        nc.vector.tensor_reduce(out=qq[:], in_=pm[:], axis=AX.X, op=ALU.add)
        lq = sb.tile([PP, 1], FP, tag="lq")
        nc.scalar.activation(out=lq[:], in_=qq[:], func=AF.Ln)
        lzF = sb.tile([PP, NRN], FP, tag="lzF")
        nc.scalar.activation(out=lzF[:], in_=zbufF[:], func=AF.Ln)
        sF = sb.tile([PP, 1], FP, tag="sF")
        nc.vector.tensor_reduce(out=sF[:], in_=lzF[:], axis=AX.X, op=ALU.add)
        lzB = sb.tile([PP, NRN], FP, tag="lzB")
        nc.scalar.activation(out=lzB[:], in_=zbufB[:], func=AF.Ln)
        sB = sb.tile([PP, 1], FP, tag="sB")
        nc.vector.tensor_reduce(out=sB[:], in_=lzB[:], axis=AX.X, op=ALU.add)
        nc.vector.tensor_tensor(out=lq[:], in0=lq[:], in1=sF[:], op=ALU.add)
        nc.vector.tensor_tensor(out=lq[:], in0=lq[:], in1=sB[:], op=ALU.add)
        nc.vector.tensor_scalar(
            out=lq[:], in0=lq[:], scalar1=CP * (NFWD + NBWD + 1), scalar2=None, op0=ALU.add
        )
        nc.sync.dma_start(
            out=scr[0:1, :],
            in_=lq[:, :].rearrange("(b n) o -> b (n o)", n=N1)[:, 0:1],
        )

        # ---------------- unary tail (as baseline) ----------------
        efl = sb.tile([K, BC], FP, tag="efl")
        nc.sync.dma_start(out=efl[:], in_=eflast[:])
        efl_e = sb.tile([K, BC], BF, tag="efl_e")
        nc.scalar.activation(out=efl_e[:], in_=efl[:], func=AF.Exp)
        ustail = sb.tile([UROW, BC], BF, tag="ustail")
        nc.vector.tensor_copy(out=ustail[:], in_=unary["stU"][:])
        usb_c = sb.tile([K, BC], BF, tag="usb_c")
        nc.sync.dma_start(out=usb_c[:], in_=ustail[32 : 32 + K, :])
        um = sb.tile([K, BC], BF, tag="umeet")
        nc.vector.tensor_tensor(out=um[:], in0=ustail[0:K, :], in1=usb_c[:], op=ALU.mult)
        nc.vector.tensor_tensor(out=um[:], in0=um[:], in1=efl_e[:], op=ALU.mult)
        ones_k = sb.tile([K, 1], BF, tag="ones_k")
        nc.vector.memset(ones_k[:], 1.0)
        au_ps = ps1.tile([1, BC], FP, tag="pmisc")
        nc.tensor.matmul(out=au_ps[:], lhsT=ones_k[:], rhs=um[:], start=True, stop=True)
        lau = sb.tile([1, BC], FP, tag="lau")
        nc.scalar.activation(out=lau[:], in_=au_ps[:], func=AF.Ln)
        NZU = SL // RU + 2
        lzU = sb.tile([2, NZU * BC], FP, tag="lzU")
        nc.scalar.activation(out=lzU[:], in_=zbufU[:, 0 : NZU * BC], func=AF.Ln)
        sU = sb.tile([2, BC], FP, tag="sU")
        nc.vector.tensor_reduce(
            out=sU[:],
            in_=lzU[:].rearrange("a (s b) -> a b s", b=BC),
            axis=AX.X,
            op=ALU.add,
        )
        su_ps = ps1.tile([1, BC], FP, tag="pmisc")
        nc.tensor.matmul(out=su_ps[:], lhsT=ones2[:], rhs=sU[:], start=True, stop=True)
        nc.vector.tensor_tensor(out=lau[:], in0=lau[:], in1=su_ps[:], op=ALU.add)
        nc.vector.tensor_scalar(
            out=lau[:], in0=lau[:], scalar1=CU * (2 * SL), scalar2=None, op0=ALU.add
        )
        nc.sync.dma_start(out=scr[1:2, :], in_=lau[:])

        # ---------------- score + output ----------------
        gv = per.tile([BC, 3 * TT + 4], FP, tag="gv")
        nc.sync.dma_start(out=gv[:], in_=gvals[:])
        sc = sb.tile([BC, 1], FP, tag="sc")
        nc.vector.tensor_reduce(out=sc[:], in_=gv[:], axis=AX.X, op=ALU.add)
        app = sb.tile([BC, 1], FP, tag="app")
        nc.sync.dma_start(out=app[:], in_=scr[0:1, :].rearrange("o b -> b o"))
        alu_ = sb.tile([BC, 1], FP, tag="alu")
        nc.sync.dma_start(out=alu_[:], in_=scr[1:2, :].rearrange("o b -> b o"))
        res = sb.tile([BC, 1], FP, tag="res")
        nc.vector.tensor_tensor(out=res[:], in0=app[:], in1=alu_[:], op=ALU.add)
        nc.vector.tensor_tensor(out=res[:], in0=res[:], in1=sc[:], op=ALU.subtract)
        nc.sync.dma_start(out=nll[:], in_=res[:].rearrange("b o -> (b o)"))

    nc.compile()
    return nc


# ======================= host-side prep =======================

def _slotify(X):
    """X [BC, S, O(out), I(in)] -> per-slot tiles.
    A-slots (even s): rows (b,o1), cols (o2, i).
    B-slots (odd s):  rows (b,i1), cols (o, i2)."""
    BC_, S, _, _ = X.shape
    out = np.empty((BC_ * N1, S, NF), np.float32)
    XA = X[:, 0::2]
    SA = XA.shape[1]
    a = XA.reshape(BC_, SA, N1, N2, K).transpose(0, 2, 1, 3, 4)
    out[:, 0::2] = a.reshape(BC_ * N1, SA, NF)
    XB = X[:, 1::2]
    SB_ = XB.shape[1]
    b = XB.reshape(BC_, SB_, K, N1, N2).transpose(0, 3, 1, 2, 4)
    out[:, 1::2] = b.reshape(BC_ * N1, SB_, NF)
    return out


def prep_core_inputs(feats, fpp, transitions, tags, b0, BC, TT):
    NFWD = TT // 2 + 1
    NBWD = TT - 1 - NFWD
    SL = TT // 2
    fe = feats[b0 : b0 + BC]          # [BC, T, K]
    fp = fpp[b0 : b0 + BC]            # [BC, T, K*K]
    tg = tags[b0 : b0 + BC]           # [BC, T]
    fp4 = fp.reshape(BC, TT, K, K)    # [b, t, n(next), p(prev)]

    # fwd chain: u' = M_t u, M = fp4[:, t], t = 0..NFWD-1 (out=n, in=p)
    fwdS = _slotify(np.ascontiguousarray(fp4[:, 0:NFWD], np.float32))
    fwdS = np.ascontiguousarray(fwdS.reshape(PP8(BC), NFWD * NF))

    # bwd chain: u' = N_s u with N = M_t^T, t = TT-2-s... t = 510..257
    tidx = (TT - 2) - np.arange(NBWD)
    Nmat = fp4[:, tidx].transpose(0, 1, 3, 2)  # [b, s, o=p, i=n]
    bwdS = _slotify(np.ascontiguousarray(Nmat, np.float32))
    bwdS = np.ascontiguousarray(bwdS.reshape(PP8(BC), NBWD * NF))

    w = fp4[:, TT - 1, STOP, :]  # [BC, K]
    winit_rep = np.ascontiguousarray(
        np.repeat(w[:, None, :], N1, axis=1).reshape(BC * N1, K), np.float32
    )

    # unary Ef table (as baseline): fwd rows slot s hold feats[t=s-1] (slot0 zero);
    # bwd rows slot s hold feats[t=TT-1-s]
    H = SL
    ftp2 = np.zeros((SL, 64, BC), np.float32)
    ftp2[1:, 0:K, :] = fe[:, 0 : H - 1].transpose(1, 2, 0)
    ftp2[:, 32 : 32 + K, :] = fe[:, TT - 1 : H - 1 : -1].transpose(1, 2, 0)
    ftp2 = np.ascontiguousarray(ftp2.transpose(1, 0, 2).reshape(64, SL * BC))
    eflast = np.ascontiguousarray(fe[:, H - 1, :].T, np.float32)  # [K, BC]

    # gold-path score operands (gather = data movement; summation on device)
    tgi = np.asarray(tg, np.int64)
    te = np.concatenate([np.full((BC, 1), START, np.int64), tgi,
                         np.full((BC, 1), STOP, np.int64)], axis=1)
    nxt, prv = te[:, 1:], te[:, :-1]
    b_ = np.arange(BC)[:, None]
    t_ = np.arange(TT)[None, :]
    gvals = np.zeros((BC, 3 * TT + 4), np.float32)
    gvals[:, 0 : TT + 1] = transitions[nxt, prv]
    gvals[:, TT + 1 : 2 * TT + 1] = np.take_along_axis(
        fe, tgi[:, :, None], axis=2)[..., 0]
    gvals[:, 2 * TT + 1 : 3 * TT + 1] = fp4[b_, np.minimum(t_, TT - 2),
                                            nxt[:, 0:TT], prv[:, 0:TT]]
    gvals[:, 3 * TT] = fp4[np.arange(BC), TT - 1, STOP, tgi[:, -1]]
    gvals[:, 3 * TT - 1] = fp4[np.arange(BC), TT - 2, nxt[:, TT - 2], prv[:, TT - 2]]

    PP = BC * N1
    selw = np.zeros((PP, N1, PP), np.float32)
    b_idx = np.arange(BC)
    for k in range(N1):
        for n1p in range(N1):
            selw[b_idx * N1 + k, k, b_idx * N1 + n1p] = 1.0
    selw = selw.reshape(PP, N1 * PP).astype(ml_dtypes.bfloat16)

    ssum = np.kron(np.eye(BC, dtype=np.float32), np.ones((N1, N1), np.float32))
    ssum = ssum.astype(ml_dtypes.bfloat16)
    ident = np.eye(PP, dtype=np.float32).astype(ml_dtypes.bfloat16)

    return {
        "fwdS": fwdS,
        "bwdS": bwdS,
        "winit_rep": winit_rep,
        "ftp2": ftp2,
        "eflast": eflast,
        "transT": np.ascontiguousarray(transitions.T, np.float32),
        "transO": np.ascontiguousarray(transitions, np.float32),
        "gvals": gvals,
        "selw": selw,
        "ssum": ssum,
        "ident": ident,
    }


def PP8(BC):
    return BC * N1


_NC_CACHE = {}


def get_nc(BC, TT, TC=9, RN=96, RU=64, A_POOL=0, B_POOL=0, PACE_MS=0.0,
           CP_ENG='none', U_ENG='dve', ESUB=12):
    key = (BC, TT, TC, RN, RU, A_POOL, B_POOL, PACE_MS, CP_ENG, U_ENG, ESUB)
    if key not in _NC_CACHE:
        _NC_CACHE[key] = build_kernel(BC=BC, TT=TT, TC=TC, RN=RN, RU=RU,
                                      A_POOL=A_POOL, B_POOL=B_POOL, PACE_MS=PACE_MS,
                                      CP_ENG=CP_ENG, U_ENG=U_ENG, ESUB=ESUB)
    return _NC_CACHE[key]


def kernel(feats, feats_pp, transitions, tags):
    feats = np.asarray(feats, np.float32)
    feats_pp = np.asarray(feats_pp, np.float32)
    transitions = np.asarray(transitions, np.float32)
    tags_np = np.asarray(tags)

    BC = B // NCORES
    nc = get_nc(BC, T)
    in_maps = [
        prep_core_inputs(feats, feats_pp, transitions, tags_np, c * BC, BC, T)
        for c in range(NCORES)
    ]
    r = run_bass_kernel_spmd(nc, in_maps, list(range(NCORES)))
    out = np.concatenate([r.results[c]["nll"] for c in range(NCORES)])
    return out.astype(np.float32)


if __name__ == "__main__":
    rng = np.random.default_rng(0)
    feats = rng.standard_normal((B, T, K), dtype=np.float32)
    fpp = rng.standard_normal((B, T, K * K), dtype=np.float32)
    tr = rng.standard_normal((K, K), dtype=np.float32)
    tr[START, :] = -100.0
    tr[:, STOP] = -100.0
    tags = rng.integers(0, K - 2, size=(B, T)).astype(np.int32)
    out = kernel(feats, fpp, tr, tags)
    print(out.shape, out[:4])


# revision 10
# speedup vs baseline: 1.0834x; 1.0062x over previous
"""DTranNER CRF loss kernel for Trainium2 (8 NeuronCores, data-parallel over batch).

v3 architecture ("alternating-layout scan"):

Batch (B=256) sharded 8 ways (32 sentences/core).  The pairwise CRF
log-partition runs as two vector chains (fwd 257 mats / bwd 254 mats) in
factored linear space.  Each chain-step alternates between two layouts:

* A-step (n-major): partitions (b, n1), free (n2, p).  DVE tensor_tensor
  (bf16, 2x mode) multiplies the exp'ed stream by the replicated state;
  DVE tensor_reduce folds p (innermost 24) -> frag [(b,n1), n2].
* B-step (p-major): partitions (b, p1), free (n, p2).  The A-step frag is
  consumed IN PLACE (each row (b,p1) already holds its own 6-slice of u);
  after the multiply, SIX accumulating PE matmuls (stationary block-selector
  lhsT, strided rhs column-slices) do BOTH the p2 reduction and the
  4-group partition sum + replication in one PSUM tile.  A scalar-engine
  copy (with folded 1/z renorm scale every RN steps) returns the state to
  SBUF bf16.

A fraction of the multiplies and state copies runs on the otherwise-idle
GpSimd (Pool) engine; the unary CRF chain's elementwise multiply also runs
on Pool, its matvec on the tensor engine.  Gold-path scores are
host-gathered operand values (pure data movement); all arithmetic happens
on device.
"""

import numpy as np
import ml_dtypes
from contextlib import ExitStack

import concourse.bass as bass
import concourse.bacc as bacc
import concourse.tile as tile
from concourse import mybir
from concourse.bass_utils import run_bass_kernel_spmd

FP = mybir.dt.float32
BF = mybir.dt.bfloat16

B, T, K = 256, 512, 24
START, STOP = 22, 23
NCORES = 8
N1, N2 = 4, 6
NF = N2 * K  # 144

AF = mybir.ActivationFunctionType
ALU = mybir.AluOpType
AX = mybir.AxisListType


def build_kernel(BC=32, TT=512, TC=32, RN=16, RU=16, A_POOL=0, B_POOL=0, PACE_MS=0.0, CP_ENG='none', U_ENG='dve', ESUB=8, SBB=3, PSB=2, BGB=3):
    """A_POOL/B_POOL of 16 A-/B-step multiplies run on Pool (rest DVE)."""
    PP = BC * N1           # 128
    NFWD = TT // 2 + 1     # 257 fwd matrices (t = 0..256)
    NBWD = TT - 1 - NFWD   # 254 bwd matrices (t = 510..257)
    SL = TT // 2           # unary slots
    UROW = 64
    CP = 3.8               # pairwise exp pre-scale (exp(x-CP))
    CU = 3.8               # unary exp pre-scale
    NRN = 18               # renorm z slots per pairwise chain

    nc = bacc.Bacc("TRN2", target_bir_lowering=False)
    fwdS = nc.dram_tensor("fwdS", [PP, NFWD * NF], FP, kind="ExternalInput")
    bwdS = nc.dram_tensor("bwdS", [PP, NBWD * NF], FP, kind="ExternalInput")
    winit_rep = nc.dram_tensor("winit_rep", [PP, K], FP, kind="ExternalInput")
    ftp2 = nc.dram_tensor("ftp2", [UROW, SL * BC], FP, kind="ExternalInput")
    eflast = nc.dram_tensor("eflast", [K, BC], FP, kind="ExternalInput")
    transT = nc.dram_tensor("transT", [K, K], FP, kind="ExternalInput")
    transO = nc.dram_tensor("transO", [K, K], FP, kind="ExternalInput")
    gvals = nc.dram_tensor("gvals", [BC, 3 * TT + 4], FP, kind="ExternalInput")
    selpack = nc.dram_tensor("selpack", [PP, N1 * PP + 2 * PP], BF, kind="ExternalInput")
    nll = nc.dram_tensor("nll", [BC], FP, kind="ExternalOutput")
    scr = nc.dram_tensor("scratch", [4, BC], FP)

    with tile.TileContext(nc) as tc, ExitStack() as ctx:
        sb = ctx.enter_context(tc.tile_pool(name="sb", bufs=SBB))
        big = ctx.enter_context(tc.tile_pool(name="big", bufs=BGB))
        ebig = ctx.enter_context(tc.tile_pool(name="ebig", bufs=BGB))
        per = ctx.enter_context(tc.tile_pool(name="per", bufs=1))
        psF = ctx.enter_context(tc.tile_pool(name="psF", bufs=PSB, space="PSUM"))
        psB = ctx.enter_context(tc.tile_pool(name="psB", bufs=PSB, space="PSUM"))
        psU = ctx.enter_context(tc.tile_pool(name="psU", bufs=2, space="PSUM"))
        ps1 = ctx.enter_context(tc.tile_pool(name="ps1", bufs=1, space="PSUM"))

        # ---------------- constants ----------------
        cpb = per.tile([128, 1], FP, tag="cpb")
        nc.vector.memset(cpb[:], -CP)
        cub = per.tile([128, 1], FP, tag="cub")
        nc.vector.memset(cub[:], -CU)
        selpk = per.tile([PP, N1 * PP + 2 * PP], BF, tag="selpk")
        nc.sync.dma_start(out=selpk[:], in_=selpack[:])
        selw_sb = selpk[:, 0 : N1 * PP]
        ssum_sb = selpk[:, N1 * PP : N1 * PP + PP]
        ident_sb = selpk[:, N1 * PP + PP : N1 * PP + 2 * PP]

        # Unary stationary weights, block matrix [UROW, UROW]
        uwst1 = per.tile([K, K], FP, tag="uwst1")
        nc.sync.dma_start(out=uwst1[:], in_=transT[:])
        uwst2 = per.tile([UROW, K], FP, tag="uwst2")
        nc.sync.dma_start(out=uwst2[32 : 32 + K, :], in_=transO[:])
        uw = per.tile([UROW, UROW], BF, tag="uw")
        nc.vector.memset(uw[:], 0.0)
        nc.scalar.activation(out=uw[0:K, 0:K], in_=uwst1[:], func=AF.Exp)
        nc.scalar.activation(
            out=uw[32 : 32 + K, 32 : 32 + K], in_=uwst2[32 : 32 + K, :], func=AF.Exp
        )

        uones = per.tile([UROW, 2], BF, tag="uones")
        nc.vector.memset(uones[:], 0.0)
        nc.vector.memset(uones[0:K, 0:1], 1.0)
        nc.vector.memset(uones[32 : 32 + K, 1:2], 1.0)
        usel = per.tile([2, UROW], BF, tag="usel")
        nc.vector.memset(usel[:], 0.0)
        nc.vector.memset(usel[0:1, 0:K], 1.0)
        rowB = sb.tile([1, UROW], BF, tag="rowB")
        nc.vector.memset(rowB[:], 0.0)
        nc.vector.memset(rowB[0:1, 32 : 32 + K], 1.0)
        nc.sync.dma_start(out=usel[1:2, :], in_=rowB[:])
        ones2 = per.tile([2, 1], FP, tag="ones2")
        nc.vector.memset(ones2[:], 1.0)

        # ---------------- unary Ef table (loaded lazily in main loop) ----
        eft = per.tile([UROW, SL * BC], BF, tag="eft")
        EFT_CHUNKS = 16
        cs2 = SL // EFT_CHUNKS
        cstep = cs2 * BC
        def load_eft_chunk(c):
            ftile = big.tile([UROW, cstep], FP, tag="ftp_in")
            nc.sync.dma_start(
                out=ftile[:], in_=ftp2[:, c * cstep : (c + 1) * cstep]
            )
            nc.scalar.activation(
                out=eft[:, c * cstep : (c + 1) * cstep], in_=ftile[:], func=AF.Exp,
                bias=cub[0:UROW, :],
            )

        # ---------------- pairwise state init ----------------
        urepF0 = per.tile([PP, K], BF, tag="urepF0")
        nc.vector.memset(urepF0[:], 0.0)
        nc.vector.memset(urepF0[:, START : START + 1], 1.0)
        wtile = sb.tile([PP, K], FP, tag="wtile")
        nc.sync.dma_start(out=wtile[:], in_=winit_rep[:])
        urepB0 = per.tile([PP, K], BF, tag="urepB0")
        nc.scalar.activation(out=urepB0[:], in_=wtile[:], func=AF.Exp, bias=cpb[0:PP, :])

        zbufF = per.tile([PP, NRN], FP, tag="zbufF")
        nc.vector.memset(zbufF[:], 1.0)
        zbufB = per.tile([PP, NRN], FP, tag="zbufB")
        nc.vector.memset(zbufB[:], 1.0)
        zbufU = per.tile([2, (SL // RU + 2) * BC], FP, tag="zbufU")
        nc.vector.memset(zbufU[:], 1.0)

        # unary state [UROW, BC]
        us0 = per.tile([UROW, BC], BF, tag="us0")
        nc.vector.memset(us0[:], 0.0)
        row1 = sb.tile([1, BC], BF, tag="row1")
        nc.vector.memset(row1[:], 1.0)
        nc.sync.dma_start(out=us0[START : START + 1, :], in_=row1[:])
        tstop = sb.tile([UROW, 1], FP, tag="tstop")
        nc.sync.dma_start(
            out=tstop[32 : 32 + K, :],
            in_=transO[STOP : STOP + 1, :].rearrange("o k -> k o"),
        )
        tstop_e = sb.tile([UROW, 1], BF, tag="tstop_e")
        nc.scalar.activation(out=tstop_e[32 : 32 + K, :], in_=tstop[32 : 32 + K, :], func=AF.Exp)
        nc.vector.tensor_copy(
            out=us0[32 : 32 + K, :], in_=tstop_e[32 : 32 + K, :].broadcast_to([K, BC])
        )
        stU = us0

        # ---------------- chain state ----------------
        stF = {"urep": urepF0, "frag": None, "rz": None}
        stB = {"urep": urepB0, "frag": None, "rz": None}
        ps_pool = {0: psF, 1: psB}
        zb = {0: zbufF, 1: zbufB}
        last_k = {0: NFWD - 1, 1: NBWD - 1}
        ntt = [0, 0]  # per-step-type TT counters

        def pairwise_step(c, st, echunk, m, k):
            e_ap = echunk[:, m * NF : (m + 1) * NF]
            if k % 2 == 0:
                # A-step: free (n2, p)
                prod = sb.tile([PP, N2, K], BF, tag="pA")
                in1 = st["urep"][:, :].unsqueeze(1).broadcast_to([PP, N2, K])
                nc.vector.tensor_tensor(
                    out=prod[:], in0=e_ap.rearrange("q (a b) -> q a b", a=N2),
                    in1=in1, op=ALU.mult,
                )
                if k == last_k[c]:
                    # final fwd A-step: reduce on DVE so the gather MMs can
                    # read frag from SBUF
                    frag = sb.tile([PP, N2], BF, tag="fr")
                    with nc.allow_low_precision("bf16 CRF inner state"):
                        nc.vector.tensor_reduce(out=frag[:], in_=prod[:], axis=AX.X, op=ALU.add)
                    if st.get("rz") is not None:
                        frag2 = sb.tile([PP, N2], BF, tag="fs")
                        nc.vector.tensor_scalar(
                            out=frag2[:], in0=frag[:], scalar1=st["rz"][:], scalar2=None,
                            op0=ALU.mult,
                        )
                        st["rz"] = None
                        frag = frag2
                    st["frag"] = frag
                else:
                    # p-reduce as 24 accumulating identity matmuls -> PSUM
                    fps = ps_pool[c].tile([PP, N2], FP, tag=f"fps{c}", bufs=1)
                    for j in range(K):
                        nc.tensor.matmul(
                            out=fps[:], lhsT=ident_sb, rhs=prod[:, :, j],
                            start=(j == 0), stop=(j == K - 1),
                        )
                    st["frag"] = fps
            else:
                # B-step: free (n, p2); reduction + group-sum via 6 PE matmuls
                prod2 = sb.tile([PP, K, N2], BF, tag="pB")
                in1 = st["frag"][:, :].unsqueeze(1).broadcast_to([PP, K, N2])
                if st.get("rz") is not None:
                    nc.vector.scalar_tensor_tensor(
                        out=prod2[:], in0=e_ap.rearrange("q (a b) -> q a b", a=K),
                        scalar=st["rz"][:], in1=in1, op0=ALU.mult, op1=ALU.mult,
                    )
                    st["rz"] = None
                else:
                    nc.vector.tensor_tensor(
                        out=prod2[:], in0=e_ap.rearrange("q (a b) -> q a b", a=K),
                        in1=in1, op=ALU.mult,
                    )
                ups = ps_pool[c].tile([PP, K], FP, tag=f"ups{c}", bufs=1)
                for j in range(N2):
                    nc.tensor.matmul(
                        out=ups[:], lhsT=ssum_sb, rhs=prod2[:, :, j],
                        start=(j == 0), stop=(j == N2 - 1),
                    )
                renorm = (k + 1) % RN == 0 and k < last_k[c]
                if renorm:
                    slot = (k + 1) // RN
                    nc.vector.tensor_reduce(
                        out=zb[c][:, slot : slot + 1], in_=ups[:], axis=AX.X, op=ALU.add
                    )
                    rz = sb.tile([PP, 1], FP, tag=f"rz{c}")
                    nc.vector.reciprocal(out=rz[:], in_=zb[c][:, slot : slot + 1])
                if CP_ENG == "none":
                    st["urep"] = ups
                    if renorm:
                        st["rz"] = rz
                else:
                    urep = sb.tile([PP, K], BF, tag="urc")
                    if renorm:
                        nc.scalar.activation(out=urep[:], in_=ups[:], func=AF.Copy, scale=rz[:])
                    elif CP_ENG == "act":
                        nc.scalar.activation(out=urep[:], in_=ups[:], func=AF.Copy)
                    else:
                        nc.vector.tensor_copy(out=urep[:], in_=ups[:])
                    st["urep"] = urep

        # ---------------- unary step ----------------
        unary = {"stU": stU, "n": 0}

        u_eng = nc.gpsimd if U_ENG == "pool" else nc.vector

        def unary_step():
            g = unary["n"]
            ef_sl = eft[:, g * BC : (g + 1) * BC]
            us_m = sb.tile([UROW, BC], BF, tag="us_m")
            u_eng.tensor_tensor(out=us_m[:], in0=unary["stU"][:], in1=ef_sl, op=ALU.mult)
            vu_ps = psU.tile([UROW, BC], FP, tag="vu")
            nc.tensor.matmul(out=vu_ps[:], lhsT=uw[:], rhs=us_m[:], start=True, stop=True)
            unary["stU"] = vu_ps
            unary["n"] += 1
            nU = unary["n"]
            if nU % RU == 0 and nU < SL:
                us_c = sb.tile([UROW, BC], BF, tag="us_c")
                nc.vector.tensor_copy(out=us_c[:], in_=vu_ps[:])
                unary["stU"] = us_c
                zu_ps = ps1.tile([2, BC], FP, tag="pmisc")
                nc.tensor.matmul(out=zu_ps[:], lhsT=uones[:], rhs=us_c[:], start=True, stop=True)
                zsl = zbufU[:, (nU // RU) * BC : (nU // RU + 1) * BC]
                nc.vector.tensor_copy(out=zsl, in_=zu_ps[:])
                rzu = sb.tile([2, BC], FP, tag="rzu")
                nc.vector.reciprocal(out=rzu[:], in_=zu_ps[:])
                rzu_b = sb.tile([2, BC], BF, tag="rzu_b")
                nc.vector.tensor_copy(out=rzu_b[:], in_=rzu[:])
                rzu_rep = ps1.tile([UROW, BC], FP, tag="pmisc")
                nc.tensor.matmul(out=rzu_rep[:], lhsT=usel[:], rhs=rzu_b[:], start=True, stop=True)
                rzu_s = sb.tile([UROW, BC], BF, tag="rzu_s")
                nc.vector.tensor_copy(out=rzu_s[:], in_=rzu_rep[:])
                us_sc = sb.tile([UROW, BC], BF, tag="us_s")
                nc.vector.tensor_tensor(out=us_sc[:], in0=us_c[:], in1=rzu_s[:], op=ALU.mult)
                unary["stU"] = us_sc

        # ---------------- main streamed loop ----------------
        load_eft_chunk(0)
        prev_eB = None
        ramp = [4, 8]
        chunk_spans = []
        pos = 0
        for r in ramp:
            chunk_spans.append((pos, r))
            pos += r
        while pos < NFWD:
            chunk_spans.append((pos, min(TC, NFWD - pos)))
            pos += TC
        eft_next = [1]
        for it, (s0, csz) in enumerate(chunk_spans):
            ntF = min(csz, NFWD - s0)
            ntB = min(ntF, NBWD - s0)
            if eft_next[0] < EFT_CHUNKS and s0 >= eft_next[0] * 12:
                load_eft_chunk(eft_next[0])
                eft_next[0] += 1
            ftileF = big.tile([PP, TC * NF], FP, tag="ftileF")
            nc.sync.dma_start(
                out=ftileF[:, 0 : ntF * NF], in_=fwdS[:, s0 * NF : (s0 + ntF) * NF]
            )
            eF = ebig.tile([PP, TC * NF], BF, tag="eF")
            if ntB > 0:
                ftileB = big.tile([PP, TC * NF], FP, tag="ftileB")
                nc.sync.dma_start(
                    out=ftileB[:, 0 : ntB * NF], in_=bwdS[:, s0 * NF : (s0 + ntB) * NF]
                )
                eB = ebig.tile([PP, TC * NF], BF, tag="eB")
            for e0 in range(0, ntF, ESUB):
                e1 = min(e0 + ESUB, ntF)
                nc.scalar.activation(
                    out=eF[:, e0 * NF : e1 * NF], in_=ftileF[:, e0 * NF : e1 * NF],
                    func=AF.Exp, bias=cpb[0:PP, :],
                )
                if ntB > 0 and e0 < ntB:
                    eb1 = min(e0 + ESUB, ntB)
                    nc.scalar.activation(
                        out=eB[:, e0 * NF : eb1 * NF], in_=ftileB[:, e0 * NF : eb1 * NF],
                        func=AF.Exp, bias=cpb[0:PP, :],
                    )
            for m in range(ntF):
                sF_i = s0 + m
                # bwd chain runs one time-step behind fwd (anti-phase: its
                # A-step overlaps fwd's PE round-trip and vice versa)
                sB_i = sF_i - 1
                with tc.tile_wait_until(sF_i * PACE_MS, enable=PACE_MS > 0):
                    if 0 <= sB_i < NBWD:
                        if m == 0:
                            pairwise_step(1, stB, prev_eB[0], prev_eB[1] - 1, sB_i)
                        else:
                            pairwise_step(1, stB, eB, m - 1, sB_i)
                    pairwise_step(0, stF, eF, m, sF_i)
                    if sF_i < SL:
                        unary_step()
            if ntB > 0:
                prev_eB = (eB, ntB)

        # ---------------- pairwise tails ----------------
        # fwd ended with the extra A-step (k=256): gather frag -> full u (4 MMs)
        upsF = ps1.tile([PP, K], FP, tag="pmisc")
        for g in range(N1):
            nc.tensor.matmul(
                out=upsF[:, g * N2 : (g + 1) * N2],
                lhsT=selw_sb[:, g * PP : (g + 1) * PP],
                rhs=stF["frag"][:], start=True, stop=True,
            )
        urepF_fin = sb.tile([PP, K], BF, tag="urf")
        nc.vector.tensor_copy(out=urepF_fin[:], in_=upsF[:])

        pm = sb.tile([PP, K], FP, tag="pmeet")
        nc.vector.tensor_tensor(out=pm[:], in0=urepF_fin[:], in1=stB["urep"][:], op=ALU.mult)
        qq = sb.tile([PP, 1], FP, tag="qq")
        nc.vector.tensor_reduce(out=qq[:], in_=pm[:], axis=AX.X, op=ALU.add)
        lq = sb.tile([PP, 1], FP, tag="lq")
        nc.scalar.activation(out=lq[:], in_=qq[:], func=AF.Ln)
        lzF = sb.tile([PP, NRN], FP, tag="lzF")
        nc.scalar.activation(out=lzF[:], in_=zbufF[:], func=AF.Ln)
        sF = sb.tile([PP, 1], FP, tag="sF")
        nc.vector.tensor_reduce(out=sF[:], in_=lzF[:], axis=AX.X, op=ALU.add)
        lzB = sb.tile([PP, NRN], FP, tag="lzB")
        nc.scalar.activation(out=lzB[:], in_=zbufB[:], func=AF.Ln)
        sB = sb.tile([PP, 1], FP, tag="sB")
        nc.vector.tensor_reduce(out=sB[:], in_=lzB[:], axis=AX.X, op=ALU.add)
        nc.vector.tensor_tensor(out=lq[:], in0=lq[:], in1=sF[:], op=ALU.add)
        nc.vector.tensor_tensor(out=lq[:], in0=lq[:], in1=sB[:], op=ALU.add)
        nc.vector.tensor_scalar(
            out=lq[:], in0=lq[:], scalar1=CP * (NFWD + NBWD + 1), scalar2=None, op0=ALU.add
        )
        nc.sync.dma_start(
            out=scr[0:1, :],
            in_=lq[:, :].rearrange("(b n) o -> b (n o)", n=N1)[:, 0:1],
        )

        # ---------------- unary tail (as baseline) ----------------
        efl = sb.tile([K, BC], FP, tag="efl")
        nc.sync.dma_start(out=efl[:], in_=eflast[:])
        efl_e = sb.tile([K, BC], BF, tag="efl_e")
        nc.scalar.activation(out=efl_e[:], in_=efl[:], func=AF.Exp)
        ustail = sb.tile([UROW, BC], BF, tag="ustail")
        nc.vector.tensor_copy(out=ustail[:], in_=unary["stU"][:])
        usb_c = sb.tile([K, BC], BF, tag="usb_c")
        nc.sync.dma_start(out=usb_c[:], in_=ustail[32 : 32 + K, :])
        um = sb.tile([K, BC], BF, tag="umeet")
        nc.vector.tensor_tensor(out=um[:], in0=ustail[0:K, :], in1=usb_c[:], op=ALU.mult)
        nc.vector.tensor_tensor(out=um[:], in0=um[:], in1=efl_e[:], op=ALU.mult)
        ones_k = sb.tile([K, 1], BF, tag="ones_k")
        nc.vector.memset(ones_k[:], 1.0)
        au_ps = ps1.tile([1, BC], FP, tag="pmisc")
        nc.tensor.matmul(out=au_ps[:], lhsT=ones_k[:], rhs=um[:], start=True, stop=True)
        lau = sb.tile([1, BC], FP, tag="lau")
        nc.scalar.activation(out=lau[:], in_=au_ps[:], func=AF.Ln)
        NZU = SL // RU + 2
        lzU = sb.tile([2, NZU * BC], FP, tag="lzU")
        nc.scalar.activation(out=lzU[:], in_=zbufU[:, 0 : NZU * BC], func=AF.Ln)
        sU = sb.tile([2, BC], FP, tag="sU")
        nc.vector.tensor_reduce(
            out=sU[:],
            in_=lzU[:].rearrange("a (s b) -> a b s", b=BC),
            axis=AX.X,
            op=ALU.add,
        )
        su_ps = ps1.tile([1, BC], FP, tag="pmisc")
        nc.tensor.matmul(out=su_ps[:], lhsT=ones2[:], rhs=sU[:], start=True, stop=True)
        nc.vector.tensor_tensor(out=lau[:], in0=lau[:], in1=su_ps[:], op=ALU.add)
        nc.vector.tensor_scalar(
            out=lau[:], in0=lau[:], scalar1=CU * (2 * SL), scalar2=None, op0=ALU.add
        )
        nc.sync.dma_start(out=scr[1:2, :], in_=lau[:])

        # ---------------- score + output ----------------
        gv = per.tile([BC, 3 * TT + 4], FP, tag="gv")
        nc.sync.dma_start(out=gv[:], in_=gvals[:])
        sc = sb.tile([BC, 1], FP, tag="sc")
        nc.vector.tensor_reduce(out=sc[:], in_=gv[:], axis=AX.X, op=ALU.add)
        app = sb.tile([BC, 1], FP, tag="app")
        nc.sync.dma_start(out=app[:], in_=scr[0:1, :].rearrange("o b -> b o"))
        alu_ = sb.tile([BC, 1], FP, tag="alu")
        nc.sync.dma_start(out=alu_[:], in_=scr[1:2, :].rearrange("o b -> b o"))
        res = sb.tile([BC, 1], FP, tag="res")
        nc.vector.tensor_tensor(out=res[:], in0=app[:], in1=alu_[:], op=ALU.add)
        nc.vector.tensor_tensor(out=res[:], in0=res[:], in1=sc[:], op=ALU.subtract)
        nc.sync.dma_start(out=nll[:], in_=res[:].rearrange("b o -> (b o)"))

    nc.compile()
    return nc


# ======================= host-side prep =======================

def _slotify(X):
    """X [BC, S, O(out), I(in)] -> per-slot tiles.
    A-slots (even s): rows (b,o1), cols (o2, i).
    B-slots (odd s):  rows (b,i1), cols (o, i2)."""
    BC_, S, _, _ = X.shape
    out = np.empty((BC_ * N1, S, NF), np.float32)
    XA = X[:, 0::2]
    SA = XA.shape[1]
    a = XA.reshape(BC_, SA, N1, N2, K).transpose(0, 2, 1, 3, 4)
    out[:, 0::2] = a.reshape(BC_ * N1, SA, NF)
    XB = X[:, 1::2]
    SB_ = XB.shape[1]
    b = XB.reshape(BC_, SB_, K, N1, N2).transpose(0, 3, 1, 2, 4)
    out[:, 1::2] = b.reshape(BC_ * N1, SB_, NF)
    return out


def prep_core_inputs(feats, fpp, transitions, tags, b0, BC, TT):
    NFWD = TT // 2 + 1
    NBWD = TT - 1 - NFWD
    SL = TT // 2
    fe = feats[b0 : b0 + BC]          # [BC, T, K]
    fp = fpp[b0 : b0 + BC]            # [BC, T, K*K]
    tg = tags[b0 : b0 + BC]           # [BC, T]
    fp4 = fp.reshape(BC, TT, K, K)    # [b, t, n(next), p(prev)]

    # fwd chain: u' = M_t u, M = fp4[:, t], t = 0..NFWD-1 (out=n, in=p)
    fwdS = _slotify(np.ascontiguousarray(fp4[:, 0:NFWD], np.float32))
    fwdS = np.ascontiguousarray(fwdS.reshape(PP8(BC), NFWD * NF))

    # bwd chain: u' = N_s u with N = M_t^T, t = TT-2-s... t = 510..257
    tidx = (TT - 2) - np.arange(NBWD)
    Nmat = fp4[:, tidx].transpose(0, 1, 3, 2)  # [b, s, o=p, i=n]
    bwdS = _slotify(np.ascontiguousarray(Nmat, np.float32))
    bwdS = np.ascontiguousarray(bwdS.reshape(PP8(BC), NBWD * NF))

    w = fp4[:, TT - 1, STOP, :]  # [BC, K]
    winit_rep = np.ascontiguousarray(
        np.repeat(w[:, None, :], N1, axis=1).reshape(BC * N1, K), np.float32
    )

    # unary Ef table (as baseline): fwd rows slot s hold feats[t=s-1] (slot0 zero);
    # bwd rows slot s hold feats[t=TT-1-s]
    H = SL
    ftp2 = np.zeros((SL, 64, BC), np.float32)
    ftp2[1:, 0:K, :] = fe[:, 0 : H - 1].transpose(1, 2, 0)
    ftp2[:, 32 : 32 + K, :] = fe[:, TT - 1 : H - 1 : -1].transpose(1, 2, 0)
    ftp2 = np.ascontiguousarray(ftp2.transpose(1, 0, 2).reshape(64, SL * BC))
    eflast = np.ascontiguousarray(fe[:, H - 1, :].T, np.float32)  # [K, BC]

    # gold-path score operands (gather = data movement; summation on device)
    tgi = np.asarray(tg, np.int64)
    te = np.concatenate([np.full((BC, 1), START, np.int64), tgi,
                         np.full((BC, 1), STOP, np.int64)], axis=1)
    nxt, prv = te[:, 1:], te[:, :-1]
    b_ = np.arange(BC)[:, None]
    t_ = np.arange(TT)[None, :]
    gvals = np.zeros((BC, 3 * TT + 4), np.float32)
    gvals[:, 0 : TT + 1] = transitions[nxt, prv]
    gvals[:, TT + 1 : 2 * TT + 1] = np.take_along_axis(
        fe, tgi[:, :, None], axis=2)[..., 0]
    gvals[:, 2 * TT + 1 : 3 * TT + 1] = fp4[b_, np.minimum(t_, TT - 2),
                                            nxt[:, 0:TT], prv[:, 0:TT]]
    gvals[:, 3 * TT] = fp4[np.arange(BC), TT - 1, STOP, tgi[:, -1]]
    gvals[:, 3 * TT - 1] = fp4[np.arange(BC), TT - 2, nxt[:, TT - 2], prv[:, TT - 2]]

    PP = BC * N1
    selw = np.zeros((PP, N1, PP), np.float32)
    b_idx = np.arange(BC)
    for k in range(N1):
        for n1p in range(N1):
            selw[b_idx * N1 + k, k, b_idx * N1 + n1p] = 1.0
    selw = selw.reshape(PP, N1 * PP).astype(ml_dtypes.bfloat16)

    ssum = np.kron(np.eye(BC, dtype=np.float32), np.ones((N1, N1), np.float32))
    ssum = ssum.astype(ml_dtypes.bfloat16)
    ident = np.eye(PP, dtype=np.float32).astype(ml_dtypes.bfloat16)
    selpack = np.concatenate([selw, ssum, ident], axis=1)

    return {
        "fwdS": fwdS,
        "bwdS": bwdS,
        "winit_rep": winit_rep,
        "ftp2": ftp2,
        "eflast": eflast,
        "transT": np.ascontiguousarray(transitions.T, np.float32),
        "transO": np.ascontiguousarray(transitions, np.float32),
        "gvals": gvals,
        "selpack": selpack,
    }


def PP8(BC):
    return BC * N1


_NC_CACHE = {}


def get_nc(BC, TT, TC=9, RN=96, RU=64, A_POOL=0, B_POOL=0, PACE_MS=0.0,
           CP_ENG='none', U_ENG='dve', ESUB=12):
    key = (BC, TT, TC, RN, RU, A_POOL, B_POOL, PACE_MS, CP_ENG, U_ENG, ESUB)
    if key not in _NC_CACHE:
        _NC_CACHE[key] = build_kernel(BC=BC, TT=TT, TC=TC, RN=RN, RU=RU,
                                      A_POOL=A_POOL, B_POOL=B_POOL, PACE_MS=PACE_MS,
                                      CP_ENG=CP_ENG, U_ENG=U_ENG, ESUB=ESUB)
    return _NC_CACHE[key]


def kernel(feats, feats_pp, transitions, tags):
    feats = np.asarray(feats, np.float32)
    feats_pp = np.asarray(feats_pp, np.float32)
    transitions = np.asarray(transitions, np.float32)
    tags_np = np.asarray(tags)

    BC = B // NCORES
    nc = get_nc(BC, T)
    in_maps = [
        prep_core_inputs(feats, feats_pp, transitions, tags_np, c * BC, BC, T)
        for c in range(NCORES)
    ]
    r = run_bass_kernel_spmd(nc, in_maps, list(range(NCORES)))
    out = np.concatenate([r.results[c]["nll"] for c in range(NCORES)])
    return out.astype(np.float32)


if __name__ == "__main__":
    rng = np.random.default_rng(0)
    feats = rng.standard_normal((B, T, K), dtype=np.float32)
    fpp = rng.standard_normal((B, T, K * K), dtype=np.float32)
    tr = rng.standard_normal((K, K), dtype=np.float32)
    tr[START, :] = -100.0
    tr[:, STOP] = -100.0
    tags = rng.integers(0, K - 2, size=(B, T)).astype(np.int32)
    out = kernel(feats, fpp, tr, tags)
    print(out.shape, out[:4])
